# revision 1
# baseline (speedup 1.0000x reference)
"""Trainium2 Bass kernel for nn_ConvWithFilter (per-location conv filters).

Computation: out[n, o, h, w] = relu( sum_k unfold(features)[n, l, k] *
filters[n, l, k, o] ),  l = h*W + w,  k in [0, C*3*3) ordered (c, kh, kw).

Strategy: filters dominate traffic (288 MiB f32). Shard (n, l-quarter)
across 8 cores -> 1024 locations/core. Host does the (cheap) unfold and a
bf16 cast; each core streams its 18 MiB bf16 filter shard through SBUF with
locations on partitions and does a fused multiply+reduce per output channel
on the Vector engine (accumulation in fp32). Output is f32.
"""

import numpy as np
import ml_dtypes

# Problem constants (hardcoded; kernel.py must be self-contained).
N, C, H, W = 2, 32, 64, 64
KSZ = 3
O = 32                 # out channels
K = C * KSZ * KSZ      # 288 contraction length
L = H * W              # 4096 locations
NCORES = 8
LSH = (N * L) // NCORES   # 1024 locations per core
P = 128                   # locations per block (SBUF partitions)
NBLK = LSH // P           # 8 blocks per core

BF16 = ml_dtypes.bfloat16

# Number of output channels whose reduce runs on the Scalar engine (the rest
# go through one strided tensor_reduce on the Vector engine). Balances the
# two engines: DVE also carries the elementwise multiply.
ACT_SPLIT = 19

# Ship the unfolded features pre-broadcast across output channels from the
# host ([LSH, K*O] bf16, rep[l, k*O+o] = feat[l, k]). The big multiply then
# reads only unit-stride bf16 operands and hits the DVE 2x perf mode,
# halving the dominant DVE op; DMA roughly doubles but stays below compute.
HOST_REP = True

TRACE = False
TRACE_KW = {}

_CACHE = {}


def _build_nc(repeat=1):
    from concourse import bacc, tile, mybir
    from contextlib import nullcontext

    nc = bacc.Bacc("TRN2", debug=False)
    dt = mybir.dt

    filt = nc.dram_tensor("filt", [LSH, K * O], dt.bfloat16, kind="ExternalInput")
    feat_shape = [LSH, K * O] if HOST_REP else [LSH, K]
    feat = nc.dram_tensor("feat", feat_shape, dt.bfloat16, kind="ExternalInput")
    out = nc.dram_tensor("out", [LSH, O], dt.float32, kind="ExternalOutput")

    filt_ap = filt.ap()
    feat_ap = feat.ap()
    out_ap = out.ap()

    n_act = ACT_SPLIT
    with tile.TileContext(nc) as tc:
        rep_ctx = tc.For_i(0, repeat, 1) if repeat > 1 else nullcontext()
        with (
            tc.tile_pool(name="filtp", bufs=3) as filtp,
            tc.tile_pool(name="featp", bufs=3) as featp,
            tc.tile_pool(name="workp", bufs=3) as workp,
            tc.tile_pool(name="outp", bufs=3) as outp,
            rep_ctx,
        ):
            for b in range(NBLK):
                rows = slice(b * P, (b + 1) * P)
                ft = filtp.tile([P, K * O], dt.bfloat16, tag="ft")
                # Two half DMAs pipeline better than one 2.25 MiB transfer.
                nc.sync.dma_start(out=ft[:, : K * O // 2], in_=filt_ap[rows, : K * O // 2])
                nc.sync.dma_start(out=ft[:, K * O // 2 :], in_=filt_ap[rows, K * O // 2 :])
                acc = workp.tile([P, O], dt.float32, tag="acc")
                scratch = workp.tile([P, K], dt.bfloat16, tag="scr")
                prod = workp.tile([P, K * O], dt.bfloat16, tag="prod")
                if HOST_REP:
                    fe = featp.tile([P, K * O], dt.bfloat16, tag="fe")
                    nc.sync.dma_start(
                        out=fe[:, : K * O // 2], in_=feat_ap[rows, : K * O // 2]
                    )
                    nc.sync.dma_start(
                        out=fe[:, K * O // 2 :], in_=feat_ap[rows, K * O // 2 :]
                    )
                    nc.vector.tensor_tensor(
                        out=prod[:], in0=ft[:], in1=fe[:],
                        op=mybir.AluOpType.mult,
                    )
                else:
                    fe = featp.tile([P, K], dt.bfloat16, tag="fe")
                    nc.sync.dma_start(out=fe[:], in_=feat_ap[rows, :])
                    nc.vector.tensor_tensor(
                        out=prod[:].rearrange("p (k o) -> p k o", o=O),
                        in0=ft[:].rearrange("p (k o) -> p k o", o=O),
                        in1=fe[:].broadcast_to((P, K, O)),
                        op=mybir.AluOpType.mult,
                    )
                if n_act < O:
                    # One strided reduce covers channels [n_act, O).
                    nc.vector.tensor_reduce(
                        out=acc[:, n_act:O],
                        in_=prod[:].rearrange("p (k o) -> p o k", o=O)[:, n_act:O, :],
                        axis=mybir.AxisListType.X,
                        op=mybir.AluOpType.add,
                    )
                for o in range(n_act):
                    nc.scalar.activation(
                        out=scratch[:],
                        in_=prod[:, o::O],
                        func=mybir.ActivationFunctionType.Copy,
                        accum_out=acc[:, o : o + 1],
                    )
                ot = outp.tile([P, O], dt.float32, tag="ot")
                nc.vector.tensor_scalar_max(out=ot[:], in0=acc[:], scalar1=0.0)
                nc.sync.dma_start(out=out_ap[rows, :], in_=ot[:])
    nc.compile()
    return nc


def _build_null_nc():
    """I/O-only kernel (for wall-clock baseline subtraction in test.py)."""
    from concourse import bacc, tile, mybir

    nc = bacc.Bacc("TRN2", debug=False)
    dt = mybir.dt
    nc.dram_tensor("filt", [LSH, K * O], dt.bfloat16, kind="ExternalInput")
    feat_shape = [LSH, K * O] if HOST_REP else [LSH, K]
    feat = nc.dram_tensor("feat", feat_shape, dt.bfloat16, kind="ExternalInput")
    out = nc.dram_tensor("out", [LSH, O], dt.float32, kind="ExternalOutput")
    with tile.TileContext(nc) as tc:
        with tc.tile_pool(name="p", bufs=2) as pool:
            for b in range(NBLK):
                rows = slice(b * P, (b + 1) * P)
                t = pool.tile([P, O], dt.bfloat16, tag="t")
                nc.sync.dma_start(out=t[:], in_=feat.ap()[rows, :O])
                ot = pool.tile([P, O], dt.float32, tag="ot")
                nc.vector.tensor_scalar_max(out=ot[:], in0=t[:], scalar1=0.0)
                nc.sync.dma_start(out=out.ap()[rows, :], in_=ot[:])
    nc.compile()
    return nc


def _unfold_np(x):
    """numpy mirror of the reference unfold: [N,C,H,W] -> [N, L, C*9]."""
    xp = np.pad(x, ((0, 0), (0, 0), (1, 1), (1, 1)))
    patches = [
        xp[:, :, i : i + H, j : j + W] for i in range(KSZ) for j in range(KSZ)
    ]
    unf = np.stack(patches, axis=2)          # [N, C, 9, H, W]
    unf = unf.reshape(N, K, L)               # k = c*9 + (kh*3+kw)
    return unf.transpose(0, 2, 1)            # [N, L, K]


def kernel(features: np.ndarray, filters: np.ndarray) -> np.ndarray:
    from concourse.bass_utils import run_bass_kernel_spmd

    features = np.asarray(features, dtype=np.float32)
    filters = np.asarray(filters, dtype=np.float32)

    feat_unf = _unfold_np(features)          # [N, L, K] f32

    in_maps = []
    for core in range(NCORES):
        n, q = divmod(core, NCORES // N)
        sl = slice(q * LSH, (q + 1) * LSH)
        fe = np.ascontiguousarray(feat_unf[n, sl]).astype(BF16)
        if HOST_REP:
            fe = np.repeat(fe, O, axis=1)  # [LSH, K*O], rep[l, k*O+o] = feat[l, k]
        in_maps.append(
            {
                "filt": np.ascontiguousarray(filters[n, sl]).reshape(LSH, K * O).astype(BF16),
                "feat": fe,
            }
        )

    if "nc" not in _CACHE:
        _CACHE["nc"] = _build_nc()
    _CACHE["in_maps"] = in_maps
    res = run_bass_kernel_spmd(
        _CACHE["nc"], in_maps, list(range(NCORES)), trace=TRACE, **TRACE_KW
    )
    _CACHE["last_result"] = res

    out = np.empty((N, O, H, W), np.float32)
    out_flat = out.reshape(N, O, L)
    for core in range(NCORES):
        n, q = divmod(core, NCORES // N)
        o = np.asarray(res.results[core]["out"], dtype=np.float32)  # [LSH, O]
        out_flat[n, :, q * LSH : (q + 1) * LSH] = o.T
    return out

